# revision 6
# baseline (speedup 1.0000x reference)
# Trainium2 Bass kernel v2 for nn_MultiHeadAttentionPure (B=2, S=1024, F=1024, H=16).
#
# Same sharding as v1 (8 cores x 16 (group,batch) attention problems, zero
# cross-core traffic), restructured on-device for engine balance:
#
#  * groups processed in QUADS of 4, one group per 32-partition strip:
#      - scores: 4 row-packed matmuls (K=16 contraction at array rows 32j)
#        into 4 PSUM banks (two [128,2,512] half-tiles) -> exp reads FD=1024
#        per ACT instruction (2 instr/tile vs 4) -- halves ACT overhead.
#      - attention-output (xt): 4 col-packed matmuls (out partitions 32j)
#        accumulate into ONE [128,512] PSUM bank; a zeroing matmul pre-sets
#        has_written for the whole bank so the 4 strips can accumulate
#        independently with start=False.
#  * renormalization: ONE reciprocal (strided partitions) + 4 gpsimd
#    partition-broadcasts + ONE [128,512] DVE multiply per (quad, chunk)
#    instead of 16 small strided ops.
#  * linear: PSUM tile [128,2,512] accumulates both 512-col halves, one
#    FD=1024 bias add, one DMA per 128 output rows.
import numpy as np

B, S, F, H = 2, 1024, 1024, 16
NG = 16          # groups per core
P = 128
NCORES = 8
NQ = 4           # quads per core


def _core_groups(c):
    b2, qq = c // 4, c % 4
    b = qq % 2
    jmod = 2 * b2 + qq // 2
    js = [4 * h2 + jmod for h2 in range(NG)]
    return b2, qq, b, js


def _build(causal: bool, n_iter: int = 1):
    import concourse.bass as bass
    import concourse.mybir as mybir
    from concourse import bacc, tile

    F32 = mybir.dt.float32
    F16 = mybir.dt.float16
    I16 = mybir.dt.int16
    INV_K1 = float(1.0 / (1024.0 / np.log(2.0)))
    AF = mybir.ActivationFunctionType
    ADD = mybir.AluOpType.add
    MUL = mybir.AluOpType.mult

    nc = bacc.Bacc("TRN2", target_bir_lowering=False, debug=False)
    qt = nc.declare_dram_parameter("qt", [NG * H, S], F16, isOutput=False)
    kt = nc.declare_dram_parameter("kt", [NG * H, S], F16, isOutput=False)
    va = nc.declare_dram_parameter("va", [S, NG * 32], F16, isOutput=False)
    wt = nc.declare_dram_parameter("wt", [F, F], F16, isOutput=False)
    msk = nc.declare_dram_parameter("msk", [P, 256], F16, isOutput=False)
    bb = nc.declare_dram_parameter("bb", [P, F], F32, isOutput=False)
    out = nc.declare_dram_parameter("o", [256, F], F32, isOutput=True)

    NT = S // P           # 8 s2 tiles

    import contextlib
    with tile.TileContext(nc) as tc:
        with tc.tile_pool(name="cst", bufs=1) as cst, \
             tc.tile_pool(name="qk", bufs=1) as qkp, \
             tc.tile_pool(name="expp", bufs=4) as expp, \
             tc.tile_pool(name="work", bufs=4) as wkp, \
             tc.tile_pool(name="yt", bufs=2) as ytp, \
             tc.tile_pool(name="stA", bufs=1, space="PSUM") as stA, \
             tc.tile_pool(name="stB", bufs=1, space="PSUM") as stB, \
             tc.tile_pool(name="xtps", bufs=2, space="PSUM") as xtps, \
             tc.tile_pool(name="xt2ps", bufs=2, space="PSUM") as xt2ps:

            # loop-invariant loads: stay OUTSIDE the For_i body
            va_sb = cst.tile([P, NT, NG * 32], F16)
            wt_sb = cst.tile([P, F // P, F], F16)
            msk_sb = cst.tile([P, 2, P], F16)
            bb_sb = cst.tile([P, F], F32)
            zrow = cst.tile([1, P], F16)
            zrs = cst.tile([1, 512], F16)
            ones_sb = cst.tile([P, 32], F16)
            nc.sync.dma_start(va_sb[:], va.rearrange("(t p) m -> p t m", p=P))
            nc.sync.dma_start(wt_sb[:], wt.rearrange("(t p) m -> p t m", p=P))
            nc.sync.dma_start(msk_sb[:], msk.rearrange("p (u m) -> p u m", u=2))
            nc.sync.dma_start(bb_sb[:], bb[:])
            nc.vector.memset(zrow[:], 0.0)
            nc.vector.memset(zrs[:], 0.0)
            nc.vector.memset(ones_sb[:], 1.0)
            qts, kts = [], []
            for Q in range(NQ):
                qt_sb = qkp.tile([P, S], F16, tag=f"qt{Q}", name=f"qt_sb{Q}")
                kt_sb = qkp.tile([P, S], F16, tag=f"kt{Q}", name=f"kt_sb{Q}")
                for j in range(4):
                    g = 4 * Q + j
                    nc.sync.dma_start(qt_sb[32 * j:32 * j + 16, :],
                                      qt[16 * g:16 * g + 16, :])
                    nc.sync.dma_start(kt_sb[32 * j:32 * j + 16, :],
                                      kt[16 * g:16 * g + 16, :])
                qts.append(qt_sb)
                kts.append(kt_sb)

            loop_ctx = tc.For_i(0, n_iter, 1, hint_engines=(
                mybir.EngineType.PE, mybir.EngineType.DVE,
                mybir.EngineType.Activation,
                mybir.EngineType.SP, mybir.EngineType.Pool,
            )) if n_iter > 1 else contextlib.nullcontext()
            with loop_ctx:
                yt_sb = ytp.tile([P, F // P, 256], F16)
                pending = []
                for Q, c in [(Q, c) for Q in range(NQ) for c in range(2)] \
                        + [(NQ, 0), (NQ, 1)]:
                    if Q == NQ:
                        while pending:
                            pending.pop(0)()
                        # linear for output rows r2=c (needs only chunk c)
                        r2 = c
                        ps = stA.tile([P, 2, 512], F32, tag="st")
                        for oc in range(2):
                            for ft in range(F // P):
                                nc.tensor.matmul(
                                    ps[:, oc, :],
                                    yt_sb[:, ft, r2 * P:(r2 + 1) * P],
                                    wt_sb[:, ft, oc * 512:(oc + 1) * 512],
                                    start=(ft == 0), stop=(ft == F // P - 1))
                        ot = wkp.tile([P, 2, 512], F32, tag="ot")
                        nc.vector.tensor_tensor(
                            out=ot[:], in0=ps[:],
                            in1=bb_sb.rearrange("p (u m) -> p u m", u=2),
                            op=ADD)
                        nc.sync.dma_start(
                            out=out[r2 * P:(r2 + 1) * P, :],
                            in_=ot.rearrange("p u m -> p (u m)"))
                        continue
                    qt_sb, kt_sb = qts[Q], kts[Q]
                    ntile = 4 * c + 4 if causal else NT
                    XT = xtps.tile([P, 512], F32, tag="xt")
                    XT2 = xt2ps.tile([P, 512], F32, tag="xt2")
                    # pre-set has_written for the whole bank so the 4
                    # col-packed strips accumulate independently below
                    nc.tensor.matmul(XT[:], zrow[:], zrs[:],
                                     start=True, stop=False)
                    nc.tensor.matmul(XT2[:], zrow[:], zrs[:],
                                     start=True, stop=False)
                    def emit_xt(pend):
                        pt, pa1, pexs = pend
                        for h in range(2):
                            for jj in range(2):
                                j = 2 * h + jj
                                g = 4 * Q + j
                                nc.tensor.matmul(
                                    XT[32 * j:32 * j + 32, pa1:],
                                    va_sb[:, pt, 32 * g:32 * g + 32],
                                    pexs[h][:, jj, pa1:],
                                    start=False, stop=False,
                                    tile_position=(0, 32 * j))
                                nc.tensor.matmul(
                                    XT2[32 * j:32 * j + 32, pa1:],
                                    ones_sb[:],
                                    pexs[h][:, jj, pa1:],
                                    start=False, stop=False,
                                    tile_position=(0, 32 * j))

                    pend = None
                    for t in range(ntile):
                        d = t - 4 * c
                        a1 = 128 * d if (causal and d >= 0) else 0
                        exs = []
                        for h in range(2):
                            pool = stA if h == 0 else stB
                            st = pool.tile([P, 2, 512], F32, tag="st")
                            for jj in range(2):
                                j = 2 * h + jj
                                nc.tensor.matmul(
                                    st[:, jj, a1:],
                                    kt_sb[32 * j:32 * j + 16, t * P:(t + 1) * P],
                                    qt_sb[32 * j:32 * j + 16,
                                          512 * c + a1:512 * (c + 1)],
                                    start=True, stop=True,
                                    tile_position=(32 * j, 0))
                            ex = expp.tile([P, 2, 512], F16, tag=f"exp{h}")
                            nc.scalar.activation(ex[:, :, a1:], st[:, :, a1:],
                                                 AF.Exp)
                            if causal and d >= 0:
                                nc.vector.tensor_tensor(
                                    out=ex[:, :, a1:a1 + P],
                                    in0=ex[:, :, a1:a1 + P],
                                    in1=msk_sb[:], op=MUL)
                            exs.append(ex)
                        # xt/den matmuls for the PREVIOUS step: keeps the PE
                        # stream from blocking scores(t+1) behind exp(t)
                        if pend is not None:
                            emit_xt(pend)
                        pend = (t, a1, exs)
                        if t == 0 and pending:
                            pending.pop(0)()
                    emit_xt(pend)
                    # close both accumulation groups: full-partition FD=1
                    # matmuls (add zero to col 0) carrying the stop flag
                    nc.tensor.matmul(XT[:, 0:1], zrow[:], zrs[:, 0:1],
                                     start=False, stop=True)
                    nc.tensor.matmul(XT2[:, 0:1], zrow[:], zrs[:, 0:1],
                                     start=False, stop=True)

                    def renorm(XT=XT, XT2=XT2, Q=Q, c=c):
                        # XT2 = denominator replicated per 32-row strip
                        recipb = wkp.tile([P, 512], F16, tag="recipb",
                                          name=f"recipb{Q}{c}")
                        with nc.allow_low_precision(reason="fp16 recip"):
                            nc.vector.reciprocal(recipb[:], XT2[:])
                        xs = wkp.tile([P, 4, P], F16, tag="xs",
                                      name=f"xs{Q}{c}")
                        for m in range(4):
                            nc.vector.tensor_tensor(out=xs[:, m, :],
                                                    in0=XT[:, m:512:4],
                                                    in1=recipb[:, m:512:4],
                                                    op=MUL)
                        for j in range(4):
                            g = 4 * Q + j
                            po = 64 * (g % 2)
                            for m in range(4):
                                nc.sync.dma_start(
                                    out=yt_sb[po + 16 * m:po + 16 * (m + 1),
                                              g // 2, 128 * c:128 * (c + 1)],
                                    in_=xs[32 * j:32 * j + 16, m, :])
                    # defer: emitted inside the NEXT chunk after its t=0
                    # masks, so this slack-rich DVE work doesn't head-of-line
                    # block the next chunk's mask multiplies
                    pending.append(renorm)
    nc.compile()
    return nc


_NC_CACHE = {}


def _get_nc(causal: bool, n_iter: int = 1):
    key = (causal, n_iter)
    if key not in _NC_CACHE:
        _NC_CACHE[key] = _build(causal, n_iter)
    return _NC_CACHE[key]


def _shard_inputs(q, k, v, W_out, b_out):
    """Build the 8 per-core input maps."""
    wt = np.ascontiguousarray(W_out.T).astype(np.float16)
    mskv = np.zeros((P, 256), np.float16)
    xi, yi = np.mgrid[0:P, 0:P]
    tri = np.where(yi >= xi, 1.0, 0.0).astype(np.float16)
    mskv[:, 0:128] = tri
    mskv[:, 128:256] = tri
    bbv = np.broadcast_to(b_out.astype(np.float32), (P, F)).copy()

    in_maps = []
    for c in range(NCORES):
        _, _, b, js = _core_groups(c)
        cols = np.concatenate([j * H + np.arange(H) for j in js])
        qc = (0.25 * q[b][:, cols].T).astype(np.float16)     # [256, S]
        kc = np.ascontiguousarray(k[b][:, cols].T).astype(np.float16)
        vav = np.zeros((S, NG, 32), np.float32)
        vav[:, :, :16] = v[b][:, cols].reshape(S, NG, H)
        in_maps.append({
            "qt": np.ascontiguousarray(qc),
            "kt": kc,
            "va": vav.reshape(S, NG * 32).astype(np.float16),
            "wt": wt,
            "msk": mskv,
            "bb": bbv,
        })
    return in_maps


def _unshard(outs):
    full = np.empty((B, S, F), np.float32)
    for c in range(NCORES):
        b2, qq, _, _ = _core_groups(c)
        full[b2, 256 * qq:256 * (qq + 1), :] = outs[c]
    return full


def _numpy_core(in_map, causal=True):
    """Numpy emulation of the device program (for host-logic validation)."""
    qt = in_map["qt"].astype(np.float32); kt = in_map["kt"].astype(np.float32)
    va = in_map["va"].reshape(S, NG, 32).astype(np.float32)
    wtm = in_map["wt"].astype(np.float32); bbv = in_map["bb"]
    ytv = np.zeros((F, 256), np.float32)
    s2i, s1i = np.mgrid[0:S, 0:S]
    for g in range(NG):
        sc = kt[g * H:(g + 1) * H].T @ qt[g * H:(g + 1) * H]   # [s2, s1]
        if causal:
            sc = np.where(s1i >= s2i, sc, -1e9)
        e = np.exp(sc).astype(np.float16).astype(np.float32)
        if causal:
            e = np.where(s1i >= s2i, e, 0.0).astype(np.float32)
        xt = va[:, g, :].T @ e                                  # [32, s1]
        den = e.sum(axis=0)
        recip = (1.0 / den).astype(np.float16).astype(np.float32)
        xs = (xt[0:16] * recip[None, :]).astype(np.float16).astype(np.float32)
        po = 64 * (g % 2)
        for m in range(4):
            for cc in range(2):
                ytv[128 * (g // 2) + po + 16 * m: 128 * (g // 2) + po + 16 * (m + 1),
                    128 * cc:128 * (cc + 1)] = xs[:, 512 * cc + m:512 * (cc + 1):4]
    o = ytv.T @ wtm + bbv[0][None, :]
    return o.astype(np.float32)


def kernel(q, k, v, W_out, b_out, apply_mask, _mock=False):
    q = np.asarray(q, np.float32)
    k = np.asarray(k, np.float32)
    v = np.asarray(v, np.float32)
    W_out = np.asarray(W_out, np.float32)
    b_out = np.asarray(b_out, np.float32)
    causal = bool(int(np.asarray(apply_mask)))
    in_maps = _shard_inputs(q, k, v, W_out, b_out)
    if _mock:
        outs = [_numpy_core(m, causal) for m in in_maps]
        return _unshard(outs)
    from concourse.bass_utils import run_bass_kernel_spmd
    nc = _get_nc(causal)
    res = run_bass_kernel_spmd(nc, in_maps, core_ids=list(range(NCORES)))
    return _unshard([r["o"] for r in res.results])
